# revision 14
# baseline (speedup 1.0000x reference)
"""GCN block (GCNConv + BN(eval) + ReLU) on 8 Trainium2 NeuronCores.

Strategy (fully data-parallel, no collectives):
  out = relu(BN(D^{-1/2}(A+I)D^{-1/2} (x W) + b))
      = relu(dis_dst * ((sum_{e->dst} xs[src] + xs[dst]) @ W') + b')
  where xs = x * dis (dis = deg^{-1/2}), W' = W * s, b' = b*s + t (BN folded).

  Nodes are sharded across 8 cores by destination block (degree-balanced
  snake deal).  The host pre-expands each core's edge source rows into a
  DENSE 1-byte-per-element fp8 stream laid out exactly as the PE wants to
  consume it ([128 slots, group, feat], edge slots grouped per 128-dst
  tile), read with plain sequential HWDGE dma_start.

  Mixed-precision selection: a fraction FRAC_DR of each tile's edge
  groups is quantized to fp8 E4M3 and reduced two-groups-per-matmul with
  MatmulPerfMode.DoubleRow (0.5 cycles/row); the rest are fp8 E3M4
  (4 mantissa bits) at 1.0 cycles/row.  Both are 1 byte/elem so the DMA
  stream stays at ~51MB/core; the fp8 rounding of dis_src (carried by
  the selection matrix) is compensated into each format's stream rows.
  FRAC_DR trades PE time against quantization noise (e2e rel err
  ~sqrt(1.34 + 3.76*frac)% vs the 2e-2 gate).

  Per 128-dst tile: edge slots are reduced into [feat, dst] PSUM via
  selection matmuls, the self-loop rows are added by the DVE during the
  PSUM->SBUF copy (tensor_tensor add with a host-pre-transposed
  [feat, dst] layout), the 512x512 transform GEMM + K=1 bias matmul
  follow, and ReLU (with the per-dst dis scale fused) writes bf16.
"""

import sys

if "/opt/trn_rl_repo" not in sys.path:
    sys.path.insert(0, "/opt/trn_rl_repo")

import math

import ml_dtypes
import numpy as np

BF16 = ml_dtypes.bfloat16
FP8E3 = ml_dtypes.float8_e3m4  # TRN FP8_EXP3 (1-3-4)
FP8E4 = ml_dtypes.float8_e4m3  # TRN FP8_EXP4 (1-4-3)

N_CORES = 8
P = 128
BN_EPS = 1e-5
TB = 8        # dst tiles per DMA batch
FRAC_DR = 0.0  # fraction of groups in e4m3 DoubleRow pairs (DR loses: stationary reload cost)


def _prep(x, edge_index, W, b, gamma, beta, running_mean, running_var):
    """Host-side preprocessing: sharding, edge slotting, dense stream
    expansion, BN folding.  Returns (meta, in_maps)."""
    N, F = x.shape
    F_OUT = W.shape[1]
    KC = F // P
    assert N % N_CORES == 0
    NB = N // N_CORES
    T = math.ceil(NB / P)  # dst tiles per core

    src = np.asarray(edge_index[0], dtype=np.int64)
    dst = np.asarray(edge_index[1], dtype=np.int64)

    deg = 1.0 + np.bincount(dst, minlength=N).astype(np.float64)
    dis = (1.0 / np.sqrt(deg)).astype(np.float32)

    xf = np.asarray(x, np.float32)
    # fp8 streams (both flavors); the fp8 rounding of dis (which rides
    # the selection matrix) is compensated into the stream quantization
    # per format so the carried product is x*dis exactly on average.
    dis83 = dis.astype(FP8E3)
    dis83f = dis83.astype(np.float32)
    x83 = (xf * (dis / dis83f)[:, None]).astype(FP8E3)
    dis84 = dis.astype(FP8E4)
    dis84f = dis84.astype(np.float32)
    x84 = (xf * (dis / dis84f)[:, None]).astype(FP8E4)
    xs = (xf * dis[:, None]).astype(np.float32)   # self-loop rows

    # BN folding into W and b.
    s = (np.asarray(gamma, np.float32)
         / np.sqrt(np.asarray(running_var, np.float32) + BN_EPS))
    t = np.asarray(beta, np.float32) - np.asarray(running_mean, np.float32) * s
    Wp = (np.asarray(W, np.float32) * s[None, :]).astype(BF16)
    bp = (np.asarray(b, np.float32) * s + t).astype(np.float32)
    wp = np.ascontiguousarray(Wp.reshape(KC, P, F_OUT).transpose(1, 0, 2))

    # ---- degree-balanced node -> (core, tile, slot) assignment (snake deal)
    NBINS = N_CORES * T
    order = np.argsort(-(deg - 1.0), kind="stable")
    assign = np.empty(N, np.int64)   # node -> bin
    slot_of = np.empty(N, np.int64)  # node -> slot within bin
    pos = 0
    rnd = 0
    while pos < N:
        chunk = order[pos:pos + NBINS]
        if rnd % 2 == 0:
            bins = np.arange(len(chunk))
        else:
            bins = NBINS - 1 - np.arange(len(chunk))
        assign[chunk] = bins
        slot_of[chunk] = rnd
        pos += NBINS
        rnd += 1
    assert rnd <= P, f"too many slot rounds {rnd}"
    core_of_bin = assign % N_CORES
    tile_of_bin = assign // N_CORES

    # node_map[k][t, p] = original node id (or -1)
    node_map = np.full((N_CORES, T, P), -1, dtype=np.int64)
    node_map[core_of_bin, tile_of_bin, slot_of] = np.arange(N)

    e_core = core_of_bin[dst]
    e_tile = tile_of_bin[dst]
    e_slot = slot_of[dst]

    # ---- pass 1: per-core edge lists sorted by tile, per-tile counts
    per_core = []
    cnt = np.zeros((N_CORES, T), dtype=np.int64)
    for k in range(N_CORES):
        m = e_core == k
        s_k = src[m]
        t_k = e_tile[m]
        p_k = e_slot[m]
        o = np.argsort(t_k, kind="stable")
        s_k, t_k, p_k = s_k[o], t_k[o], p_k[o]
        bounds = np.searchsorted(t_k, np.arange(T + 1))
        cnt[k] = bounds[1:] - bounds[:-1]
        per_core.append((s_k, p_k, bounds))

    S_t = (np.ceil(cnt.max(axis=0) / P).astype(np.int64) * P)
    S_t = np.maximum(S_t, P)
    off_t = np.concatenate([[0], np.cumsum(S_t)])
    TOT = int(off_t[-1])
    NG_t = (S_t // P).astype(np.int64)
    G_off = (off_t // P).astype(np.int64)
    G_TOT = TOT // P

    # DoubleRow group counts per tile (even, <= NG_t)
    NGD_t = np.minimum((np.round(FRAC_DR * NG_t / 2) * 2).astype(np.int64),
                       NG_t - (NG_t % 2 == 1))
    NGD_t = np.maximum(NGD_t, 0)
    # per-group DR flag
    g_is_dr = np.zeros(G_TOT, dtype=bool)
    for tt in range(T):
        g_is_dr[G_off[tt]:G_off[tt] + NGD_t[tt]] = True
    slot_is_dr = np.repeat(g_is_dr, P)  # [TOT]

    x83b = x83.view(np.uint8)
    x84b = x84.view(np.uint8)
    d83b = dis83.view(np.uint8)
    d84b = dis84.view(np.uint8)

    # ---- pass 2: per-core arrays
    in_maps = []
    for k in range(N_CORES):
        s_k, p_k, bounds = per_core[k]
        srcs_flat = np.zeros(TOT, dtype=np.int64)
        dstl_flat = np.full(TOT, -1.0, dtype=np.float32)
        for tt in range(T):
            t_lo, t_hi = bounds[tt], bounds[tt + 1]
            n_e = t_hi - t_lo
            o = off_t[tt]
            srcs_flat[o:o + n_e] = s_k[t_lo:t_hi]
            dstl_flat[o:o + n_e] = p_k[t_lo:t_hi].astype(np.float32)
        # dense expanded stream (raw fp8 bytes, format per group)
        sb = np.where(slot_is_dr[:, None], x84b[srcs_flat], x83b[srcs_flat])
        stream = np.ascontiguousarray(
            sb.reshape(G_TOT, P, F).transpose(1, 0, 2)).view(FP8E3)
        # selection matrices: sel[p, g, d] = fp8(dis_src) iff edge slot
        # g*128+p has dst slot d (0 otherwise / padding); raw bytes with
        # the format matching the group's stream format.
        oh = (dstl_flat[:, None] == np.arange(P, dtype=np.float32)[None, :]
              ).astype(np.uint8)
        vbyte = np.where(slot_is_dr, d84b[srcs_flat], d83b[srcs_flat])
        selb = oh * vbyte[:, None]
        sel = np.ascontiguousarray(
            selb.reshape(G_TOT, P, P).transpose(1, 0, 2)).view(FP8E3)

        nm = node_map[k]  # [T, P]
        valid = nm >= 0
        nm_safe = np.where(valid, nm, 0)
        dis_tp = np.where(valid, dis[nm_safe], 1.0).astype(np.float32)  # [T, P]
        dis_t = np.ascontiguousarray(dis_tp.T)  # [128, T]
        invdis_t = np.ascontiguousarray(
            np.where(valid, 1.0 / np.maximum(dis_tp, 1e-9), 0.0).T
        ).astype(np.float32)  # [128, T]
        # self-loop rows pre-transposed to the aggT layout:
        # xsoT[p, t, c*128 + d] = xs[node(t, d), c*128 + p]
        xso_rows = np.where(valid[:, :, None], xs[nm_safe], 0.0)  # [T, P(d), F]
        xsoT = np.ascontiguousarray(
            xso_rows.reshape(T, P, KC, P).transpose(3, 0, 2, 1)
            .reshape(P, T, KC * P)).astype(BF16)
        in_maps.append({
            "stream": stream,
            "sel": sel,
            "dis_t": dis_t,
            "invdis_t": invdis_t,
            "xsoT": xsoT,
            "wp": wp,
            "bp": np.ascontiguousarray(
                np.broadcast_to(bp.astype(BF16), (P, F_OUT))),
        })

    meta = {
        "N": N, "F": F, "F_OUT": F_OUT, "KC": KC, "NB": NB, "T": T,
        "TOT": TOT, "G_TOT": G_TOT,
        "NG_t": NG_t.tolist(), "NGD_t": NGD_t.tolist(),
        "G_off": G_off.tolist(),
        "node_map": node_map,
    }
    return meta, in_maps


def _build_program(meta):
    """Emit the Bass/Tile program (shared by all cores)."""
    import concourse.bacc as bacc
    import concourse.mybir as mybir
    import concourse.tile as tile

    F, F_OUT, KC = meta["F"], meta["F_OUT"], meta["KC"]
    T, G_TOT = meta["T"], meta["G_TOT"]
    NG_t, NGD_t, G_off = meta["NG_t"], meta["NGD_t"], meta["G_off"]

    dt = mybir.dt
    DR = mybir.MatmulPerfMode.DoubleRow
    nc = bacc.Bacc("TRN2", target_bir_lowering=False, debug=False,
                   enable_asserts=False, num_devices=N_CORES,
                   num_swdge_queues=4)

    stream = nc.dram_tensor("stream", [P, G_TOT, F], dt.float8e3, kind="ExternalInput").ap()
    sel = nc.dram_tensor("sel", [P, G_TOT, P], dt.float8e3, kind="ExternalInput").ap()
    dis_t = nc.dram_tensor("dis_t", [P, T], dt.float32, kind="ExternalInput").ap()
    invdis_t = nc.dram_tensor("invdis_t", [P, T], dt.float32, kind="ExternalInput").ap()
    xsoT = nc.dram_tensor("xsoT", [P, T, KC * P], dt.bfloat16, kind="ExternalInput").ap()
    wp = nc.dram_tensor("wp", [P, KC, F_OUT], dt.bfloat16, kind="ExternalInput").ap()
    bp = nc.dram_tensor("bp", [P, F_OUT], dt.bfloat16, kind="ExternalInput").ap()
    out = nc.dram_tensor("out", [P, T, F_OUT], dt.bfloat16, kind="ExternalOutput").ap()

    batches = []
    t0 = 0
    sizes = [1, 2, 4]
    while t0 < T:
        sz = sizes.pop(0) if sizes else TB
        batches.append((t0, min(t0 + sz, T)))
        t0 += sz
    max_bw = max(G_off[b1] - G_off[b0] for b0, b1 in batches)

    with tile.TileContext(nc) as tc:
        with (
            tc.tile_pool(name="const", bufs=1) as cpool,
            tc.tile_pool(name="gbuf", bufs=3) as gpool,
            tc.tile_pool(name="sel8", bufs=3) as s8pool,
            tc.tile_pool(name="xso", bufs=2) as xpool,
            tc.tile_pool(name="aggT", bufs=3) as aggpool,
            tc.tile_pool(name="outsb", bufs=2) as opool,
            tc.tile_pool(name="obuf", bufs=3) as obpool,
            tc.tile_pool(name="psA", bufs=4, space="PSUM") as psA,
            tc.tile_pool(name="psB", bufs=4, space="PSUM") as psB,
        ):
            # resident constants (tiles allocated up front; DMAs issued
            # after the first batch's input DMAs -- constants are first
            # needed only at the first emit_tail, one tile later)
            dis_sb = cpool.tile([P, T], dt.float32, tag="dis")
            invdis_sb = cpool.tile([P, T], dt.float32, tag="invdis")
            wp_sb = cpool.tile([P, KC, F_OUT], dt.bfloat16, tag="wp")
            bp_sb = cpool.tile([P, F_OUT], dt.bfloat16, tag="bp")

            def emit_consts():
                nc.sync.dma_start(dis_sb[:], dis_t[:])
                nc.sync.dma_start(invdis_sb[:], invdis_t[:])
                nc.sync.dma_start(wp_sb[:], wp[:])
                nc.sync.dma_start(bp_sb[:], bp[:])

            def emit_tail(t, aggT_sb, out_blk, trel, flush):
                """Transform GEMM + bias + ReLU for tile t (+ the batch's
                out DMA when t closes a batch).

                Emitted one tile LATE (software pipelining) so the PE's
                transform never stalls on the DVE add of the same tile:
                the PE runs [sel t, transform t-1, sel t+1, ...] while the
                DVE add of tile t overlaps with transform t-1.  Out DMAs
                go through the idle GpSimd queue so the Sync engine only
                issues input DMAs.
                """
                out_ps = psB.tile([P, F_OUT], dt.float32, tag="out_ps")
                for c in range(KC):
                    nc.tensor.matmul(
                        out_ps[:],
                        lhsT=aggT_sb[:, c * P:(c + 1) * P],
                        rhs=wp_sb[:, c, :],
                        start=(c == 0),
                        stop=(c == KC - 1),
                    )
                # bias (rank-1: bp x 1/dis) added on the DVE
                ob = obpool.tile([P, F_OUT], dt.float32, tag="ob")
                nc.vector.scalar_tensor_tensor(
                    ob[:],
                    bp_sb[:],
                    invdis_sb[:, t:t + 1],
                    out_ps[:],
                    mybir.AluOpType.mult,
                    mybir.AluOpType.add,
                )
                nc.scalar.activation(
                    out_blk[:, trel, :],
                    ob[:],
                    mybir.ActivationFunctionType.Relu,
                    scale=dis_sb[:, t:t + 1],
                )
                if flush is not None:
                    f0, f1 = flush
                    nc.gpsimd.dma_start(out[:, f0:f1, :],
                                        out_blk[:, :f1 - f0, :])

            prev = None
            for bi, (t0, t1) in enumerate(batches):
                nb_t = t1 - t0
                sg0, sg1 = G_off[t0], G_off[t1]

                g_sb = gpool.tile([P, max_bw, F], dt.float8e3, tag="g")
                self8_sb = s8pool.tile([P, max_bw, P], dt.float8e3, tag="sel8")
                # first batches split per tile so PE starts sooner
                if bi == 0:
                    for t in range(t0, t1):
                        ga, gb = G_off[t] - sg0, G_off[t + 1] - sg0
                        nc.sync.dma_start(self8_sb[:, ga:gb, :],
                                          sel[:, sg0 + ga:sg0 + gb, :])
                        nc.sync.dma_start(g_sb[:, ga:gb, :],
                                          stream[:, sg0 + ga:sg0 + gb, :])
                else:
                    nc.sync.dma_start(g_sb[:, :sg1 - sg0, :],
                                      stream[:, sg0:sg1, :])
                    nc.sync.dma_start(self8_sb[:, :sg1 - sg0, :],
                                      sel[:, sg0:sg1, :])
                xso_sb = xpool.tile([P, TB, KC * P], dt.bfloat16, tag="xso")
                nc.gpsimd.dma_start(xso_sb[:, :nb_t, :], xsoT[:, t0:t1, :])
                if bi == 0:
                    emit_consts()
                out_blk = opool.tile([P, TB, F_OUT], dt.bfloat16, tag="out_sb")

                for t in range(t0, t1):
                    ng = NG_t[t]
                    ngd = NGD_t[t]
                    goff = G_off[t] - sg0

                    # selection: aggT[fchunk, dst] += G_chunk^T @ selR
                    aggT_ps = psA.tile([P, F], dt.float32, tag="aggT_ps")
                    # e4m3 DoubleRow pairs (two groups per matmul)
                    for dpair in range(ngd // 2):
                        ga = goff + 2 * dpair
                        for c in range(KC):
                            nc.tensor.matmul(
                                aggT_ps[:, c * P:(c + 1) * P],
                                lhsT=g_sb[:, ga:ga + 2, c * P:(c + 1) * P]
                                    .bitcast(dt.float8e4),
                                rhs=self8_sb[:, ga:ga + 2, :]
                                    .bitcast(dt.float8e4),
                                start=(dpair == 0 and c == 0),
                                stop=(ng == ngd and dpair == ngd // 2 - 1
                                      and c == KC - 1),
                                perf_mode=DR,
                                skip_group_check=True,
                            )
                    # e3m4 singles
                    for g in range(ngd, ng):
                        for c in range(KC):
                            nc.tensor.matmul(
                                aggT_ps[:, c * P:(c + 1) * P],
                                lhsT=g_sb[:, goff + g, c * P:(c + 1) * P],
                                rhs=self8_sb[:, goff + g, :],
                                start=(ngd == 0 and g == 0 and c == 0),
                                stop=(g == ng - 1 and c == KC - 1),
                                skip_group_check=True,
                            )

                    # PSUM -> SBUF copy with the self-loop term fused in
                    aggT_sb = aggpool.tile([P, F], dt.bfloat16, tag="aggT_sb")
                    nc.vector.tensor_tensor(
                        aggT_sb[:],
                        aggT_ps[:],
                        xso_sb[:, t - t0, :],
                        mybir.AluOpType.add,
                    )

                    if prev is not None:
                        emit_tail(*prev)
                    prev = (t, aggT_sb, out_blk, t - t0,
                            (t0, t1) if t == t1 - 1 else None)

            emit_tail(*prev)

    nc.compile()
    return nc


_CACHE = {}


def _get_program(meta):
    key = (meta["N"], meta["F"], meta["F_OUT"], meta["TOT"], meta["G_TOT"],
           tuple(meta["NG_t"]), tuple(meta["NGD_t"]))
    if key not in _CACHE:
        _CACHE[key] = _build_program(meta)
    return _CACHE[key]


def kernel(x, edge_index, W, b, gamma, beta, running_mean, running_var,
           _want_results_holder=None, _run_kwargs=None):
    meta, in_maps = _prep(x, edge_index, W, b, gamma, beta,
                          running_mean, running_var)
    nc = _get_program(meta)

    from concourse.bass_utils import run_bass_kernel_spmd

    res = run_bass_kernel_spmd(nc, in_maps, core_ids=list(range(N_CORES)),
                               **(_run_kwargs or {}))
    if _want_results_holder is not None:
        _want_results_holder.append((nc, meta, in_maps, res))

    T, F_OUT = meta["T"], meta["F_OUT"]
    node_map = meta["node_map"]
    out = np.empty((meta["N"], F_OUT), dtype=np.float32)
    for k in range(N_CORES):
        tiled = np.asarray(res.results[k]["out"], dtype=np.float32)  # [128, T, F_OUT]
        rows = np.ascontiguousarray(tiled.transpose(1, 0, 2))  # [T, 128, F]
        nm = node_map[k]
        valid = nm >= 0
        out[nm[valid]] = rows[valid]
    return out


# revision 16
# speedup vs baseline: 1.0463x; 1.0463x over previous
"""GCN block (GCNConv + BN(eval) + ReLU) on 8 Trainium2 NeuronCores.

Strategy (fully data-parallel, no collectives):
  out = relu(BN(D^{-1/2}(A+I)D^{-1/2} (x W) + b))
      = relu(dis_dst * ((sum_{e->dst} xs[src] + xs[dst]) @ W') + b')
  where xs = x * dis (dis = deg^{-1/2}), W' = W * s, b' = b*s + t (BN folded).

  Nodes are sharded across 8 cores by destination block (degree-balanced
  snake deal).  The host pre-expands each core's edge source rows into a
  DENSE 1-byte-per-element fp8 stream laid out exactly as the PE wants to
  consume it ([128 slots, group, feat], edge slots grouped per 128-dst
  tile), read with plain sequential HWDGE dma_start.

  Mixed-precision selection: a fraction FRAC_DR of each tile's edge
  groups is quantized to fp8 E4M3 and reduced two-groups-per-matmul with
  MatmulPerfMode.DoubleRow (0.5 cycles/row); the rest are fp8 E3M4
  (4 mantissa bits) at 1.0 cycles/row.  Both are 1 byte/elem so the DMA
  stream stays at ~51MB/core; the fp8 rounding of dis_src (carried by
  the selection matrix) is compensated into each format's stream rows.
  FRAC_DR trades PE time against quantization noise (e2e rel err
  ~sqrt(1.34 + 3.76*frac)% vs the 2e-2 gate).

  Per 128-dst tile: edge slots are reduced into [feat, dst] PSUM via
  selection matmuls, the self-loop rows are added by the DVE during the
  PSUM->SBUF copy (tensor_tensor add with a host-pre-transposed
  [feat, dst] layout), the 512x512 transform GEMM + K=1 bias matmul
  follow, and ReLU (with the per-dst dis scale fused) writes bf16.
"""

import sys

if "/opt/trn_rl_repo" not in sys.path:
    sys.path.insert(0, "/opt/trn_rl_repo")

import math

import ml_dtypes
import numpy as np

BF16 = ml_dtypes.bfloat16
FP8E3 = ml_dtypes.float8_e3m4  # TRN FP8_EXP3 (1-3-4)
FP8E4 = ml_dtypes.float8_e4m3  # TRN FP8_EXP4 (1-4-3)

N_CORES = 8
P = 128
BN_EPS = 1e-5
TB = 6        # dst tiles per DMA batch
FRAC_DR = 0.0  # fraction of groups in e4m3 DoubleRow pairs (DR loses: stationary reload cost)


def _prep(x, edge_index, W, b, gamma, beta, running_mean, running_var):
    """Host-side preprocessing: sharding, edge slotting, dense stream
    expansion, BN folding.  Returns (meta, in_maps)."""
    N, F = x.shape
    F_OUT = W.shape[1]
    KC = F // P
    assert N % N_CORES == 0
    NB = N // N_CORES
    T = math.ceil(NB / P)  # dst tiles per core

    src = np.asarray(edge_index[0], dtype=np.int64)
    dst = np.asarray(edge_index[1], dtype=np.int64)

    deg = 1.0 + np.bincount(dst, minlength=N).astype(np.float64)
    dis = (1.0 / np.sqrt(deg)).astype(np.float32)

    xf = np.asarray(x, np.float32)
    # fp8 streams (both flavors); the fp8 rounding of dis (which rides
    # the selection matrix) is compensated into the stream quantization
    # per format so the carried product is x*dis exactly on average.
    dis83 = dis.astype(FP8E3)
    dis83f = dis83.astype(np.float32)
    x83 = (xf * (dis / dis83f)[:, None]).astype(FP8E3)
    dis84 = dis.astype(FP8E4)
    dis84f = dis84.astype(np.float32)
    x84 = (xf * (dis / dis84f)[:, None]).astype(FP8E4)
    xs = (xf * dis[:, None]).astype(np.float32)   # self-loop rows

    # BN folding into W and b.
    s = (np.asarray(gamma, np.float32)
         / np.sqrt(np.asarray(running_var, np.float32) + BN_EPS))
    t = np.asarray(beta, np.float32) - np.asarray(running_mean, np.float32) * s
    Wp = (np.asarray(W, np.float32) * s[None, :]).astype(BF16)
    bp = (np.asarray(b, np.float32) * s + t).astype(np.float32)
    wp = np.ascontiguousarray(Wp.reshape(KC, P, F_OUT).transpose(1, 0, 2))

    # ---- degree-balanced node -> (core, tile, slot) assignment (snake deal)
    NBINS = N_CORES * T
    order = np.argsort(-(deg - 1.0), kind="stable")
    assign = np.empty(N, np.int64)   # node -> bin
    slot_of = np.empty(N, np.int64)  # node -> slot within bin
    pos = 0
    rnd = 0
    while pos < N:
        chunk = order[pos:pos + NBINS]
        if rnd % 2 == 0:
            bins = np.arange(len(chunk))
        else:
            bins = NBINS - 1 - np.arange(len(chunk))
        assign[chunk] = bins
        slot_of[chunk] = rnd
        pos += NBINS
        rnd += 1
    assert rnd <= P, f"too many slot rounds {rnd}"
    core_of_bin = assign % N_CORES
    tile_of_bin = assign // N_CORES

    # node_map[k][t, p] = original node id (or -1)
    node_map = np.full((N_CORES, T, P), -1, dtype=np.int64)
    node_map[core_of_bin, tile_of_bin, slot_of] = np.arange(N)

    e_core = core_of_bin[dst]
    e_tile = tile_of_bin[dst]
    e_slot = slot_of[dst]

    # ---- pass 1: per-core edge lists sorted by tile, per-tile counts
    per_core = []
    cnt = np.zeros((N_CORES, T), dtype=np.int64)
    for k in range(N_CORES):
        m = e_core == k
        s_k = src[m]
        t_k = e_tile[m]
        p_k = e_slot[m]
        o = np.argsort(t_k, kind="stable")
        s_k, t_k, p_k = s_k[o], t_k[o], p_k[o]
        bounds = np.searchsorted(t_k, np.arange(T + 1))
        cnt[k] = bounds[1:] - bounds[:-1]
        per_core.append((s_k, p_k, bounds))

    S_t = (np.ceil(cnt.max(axis=0) / P).astype(np.int64) * P)
    S_t = np.maximum(S_t, P)
    off_t = np.concatenate([[0], np.cumsum(S_t)])
    TOT = int(off_t[-1])
    NG_t = (S_t // P).astype(np.int64)
    G_off = (off_t // P).astype(np.int64)
    G_TOT = TOT // P

    # DoubleRow group counts per tile (even, <= NG_t)
    NGD_t = np.minimum((np.round(FRAC_DR * NG_t / 2) * 2).astype(np.int64),
                       NG_t - (NG_t % 2 == 1))
    NGD_t = np.maximum(NGD_t, 0)
    # per-group DR flag
    g_is_dr = np.zeros(G_TOT, dtype=bool)
    for tt in range(T):
        g_is_dr[G_off[tt]:G_off[tt] + NGD_t[tt]] = True
    slot_is_dr = np.repeat(g_is_dr, P)  # [TOT]

    x83b = x83.view(np.uint8)
    x84b = x84.view(np.uint8)
    d83b = dis83.view(np.uint8)
    d84b = dis84.view(np.uint8)

    # ---- pass 2: per-core arrays
    in_maps = []
    for k in range(N_CORES):
        s_k, p_k, bounds = per_core[k]
        srcs_flat = np.zeros(TOT, dtype=np.int64)
        dstl_flat = np.full(TOT, -1.0, dtype=np.float32)
        for tt in range(T):
            t_lo, t_hi = bounds[tt], bounds[tt + 1]
            n_e = t_hi - t_lo
            o = off_t[tt]
            srcs_flat[o:o + n_e] = s_k[t_lo:t_hi]
            dstl_flat[o:o + n_e] = p_k[t_lo:t_hi].astype(np.float32)
        # dense expanded stream (raw fp8 bytes, format per group)
        sb = np.where(slot_is_dr[:, None], x84b[srcs_flat], x83b[srcs_flat])
        stream = np.ascontiguousarray(
            sb.reshape(G_TOT, P, F).transpose(1, 0, 2)).view(FP8E3)
        # selection matrices: sel[p, g, d] = fp8(dis_src) iff edge slot
        # g*128+p has dst slot d (0 otherwise / padding); raw bytes with
        # the format matching the group's stream format.
        oh = (dstl_flat[:, None] == np.arange(P, dtype=np.float32)[None, :]
              ).astype(np.uint8)
        vbyte = np.where(slot_is_dr, d84b[srcs_flat], d83b[srcs_flat])
        selb = oh * vbyte[:, None]
        sel = np.ascontiguousarray(
            selb.reshape(G_TOT, P, P).transpose(1, 0, 2)).view(FP8E3)

        nm = node_map[k]  # [T, P]
        valid = nm >= 0
        nm_safe = np.where(valid, nm, 0)
        dis_tp = np.where(valid, dis[nm_safe], 1.0).astype(np.float32)  # [T, P]
        dis_t = np.ascontiguousarray(dis_tp.T)  # [128, T]
        invdis_t = np.ascontiguousarray(
            np.where(valid, 1.0 / np.maximum(dis_tp, 1e-9), 0.0).T
        ).astype(np.float32)  # [128, T]
        # self-loop rows pre-transposed to the aggT layout:
        # xsoT[p, t, c*128 + d] = xs[node(t, d), c*128 + p]
        xso_rows = np.where(valid[:, :, None], xs[nm_safe], 0.0)  # [T, P(d), F]
        xsoT = np.ascontiguousarray(
            xso_rows.reshape(T, P, KC, P).transpose(3, 0, 2, 1)
            .reshape(P, T, KC * P)).astype(BF16)
        in_maps.append({
            "stream": stream,
            "sel": sel,
            "dis_t": dis_t,
            "invdis_t": invdis_t,
            "xsoT": xsoT,
            "wp": wp,
            "bp": np.ascontiguousarray(
                np.broadcast_to(bp.astype(BF16), (P, F_OUT))),
        })

    meta = {
        "N": N, "F": F, "F_OUT": F_OUT, "KC": KC, "NB": NB, "T": T,
        "TOT": TOT, "G_TOT": G_TOT,
        "NG_t": NG_t.tolist(), "NGD_t": NGD_t.tolist(),
        "G_off": G_off.tolist(),
        "node_map": node_map,
    }
    return meta, in_maps


def _build_program(meta):
    """Emit the Bass/Tile program (shared by all cores)."""
    import concourse.bacc as bacc
    import concourse.mybir as mybir
    import concourse.tile as tile

    F, F_OUT, KC = meta["F"], meta["F_OUT"], meta["KC"]
    T, G_TOT = meta["T"], meta["G_TOT"]
    NG_t, NGD_t, G_off = meta["NG_t"], meta["NGD_t"], meta["G_off"]

    dt = mybir.dt
    DR = mybir.MatmulPerfMode.DoubleRow
    nc = bacc.Bacc("TRN2", target_bir_lowering=False, debug=False,
                   enable_asserts=False, num_devices=N_CORES,
                   num_swdge_queues=4)

    stream = nc.dram_tensor("stream", [P, G_TOT, F], dt.float8e3, kind="ExternalInput").ap()
    sel = nc.dram_tensor("sel", [P, G_TOT, P], dt.float8e3, kind="ExternalInput").ap()
    dis_t = nc.dram_tensor("dis_t", [P, T], dt.float32, kind="ExternalInput").ap()
    invdis_t = nc.dram_tensor("invdis_t", [P, T], dt.float32, kind="ExternalInput").ap()
    xsoT = nc.dram_tensor("xsoT", [P, T, KC * P], dt.bfloat16, kind="ExternalInput").ap()
    wp = nc.dram_tensor("wp", [P, KC, F_OUT], dt.bfloat16, kind="ExternalInput").ap()
    bp = nc.dram_tensor("bp", [P, F_OUT], dt.bfloat16, kind="ExternalInput").ap()
    out = nc.dram_tensor("out", [P, T, F_OUT], dt.bfloat16, kind="ExternalOutput").ap()

    batches = []
    t0 = 0
    sizes = [1, 2, 4]
    while t0 < T:
        sz = sizes.pop(0) if sizes else TB
        batches.append((t0, min(t0 + sz, T)))
        t0 += sz
    max_bw = max(G_off[b1] - G_off[b0] for b0, b1 in batches)

    with tile.TileContext(nc) as tc:
        with (
            tc.tile_pool(name="const", bufs=1) as cpool,
            tc.tile_pool(name="gbuf", bufs=3) as gpool,
            tc.tile_pool(name="sel8", bufs=3) as s8pool,
            tc.tile_pool(name="xso", bufs=2) as xpool,
            tc.tile_pool(name="aggT", bufs=3) as aggpool,
            tc.tile_pool(name="outsb", bufs=2) as opool,
            tc.tile_pool(name="obuf", bufs=3) as obpool,
            tc.tile_pool(name="psA", bufs=4, space="PSUM") as psA,
            tc.tile_pool(name="psB", bufs=4, space="PSUM") as psB,
        ):
            # resident constants (tiles allocated up front; DMAs issued
            # after the first batch's input DMAs -- constants are first
            # needed only at the first emit_tail, one tile later)
            dis_sb = cpool.tile([P, T], dt.float32, tag="dis")
            invdis_sb = cpool.tile([P, T], dt.float32, tag="invdis")
            wp_sb = cpool.tile([P, KC, F_OUT], dt.bfloat16, tag="wp")
            bp_sb = cpool.tile([P, F_OUT], dt.bfloat16, tag="bp")

            def emit_consts():
                nc.sync.dma_start(dis_sb[:], dis_t[:])
                nc.sync.dma_start(invdis_sb[:], invdis_t[:])
                nc.sync.dma_start(wp_sb[:], wp[:])
                nc.sync.dma_start(bp_sb[:], bp[:])

            def emit_tail(t, aggT_sb, out_blk, trel, flush):
                """Transform GEMM + bias + ReLU for tile t (+ the batch's
                out DMA when t closes a batch).

                Emitted one tile LATE (software pipelining) so the PE's
                transform never stalls on the DVE add of the same tile:
                the PE runs [sel t, transform t-1, sel t+1, ...] while the
                DVE add of tile t overlaps with transform t-1.  Out DMAs
                go through the idle GpSimd queue so the Sync engine only
                issues input DMAs.
                """
                out_ps = psB.tile([P, F_OUT], dt.float32, tag="out_ps")
                for c in range(KC):
                    nc.tensor.matmul(
                        out_ps[:],
                        lhsT=aggT_sb[:, c * P:(c + 1) * P],
                        rhs=wp_sb[:, c, :],
                        start=(c == 0),
                        stop=(c == KC - 1),
                    )
                # bias (rank-1: bp x 1/dis) added on the DVE
                ob = obpool.tile([P, F_OUT], dt.float32, tag="ob")
                nc.vector.scalar_tensor_tensor(
                    ob[:],
                    bp_sb[:],
                    invdis_sb[:, t:t + 1],
                    out_ps[:],
                    mybir.AluOpType.mult,
                    mybir.AluOpType.add,
                )
                nc.scalar.activation(
                    out_blk[:, trel, :],
                    ob[:],
                    mybir.ActivationFunctionType.Relu,
                    scale=dis_sb[:, t:t + 1],
                )
                if flush is not None:
                    f0, f1 = flush
                    nc.gpsimd.dma_start(out[:, f0:f1, :],
                                        out_blk[:, :f1 - f0, :])

            prev = None
            for bi, (t0, t1) in enumerate(batches):
                nb_t = t1 - t0
                sg0, sg1 = G_off[t0], G_off[t1]

                g_sb = gpool.tile([P, max_bw, F], dt.float8e3, tag="g")
                self8_sb = s8pool.tile([P, max_bw, P], dt.float8e3, tag="sel8")
                # first batches split per tile so PE starts sooner
                if bi == 0:
                    for t in range(t0, t1):
                        ga, gb = G_off[t] - sg0, G_off[t + 1] - sg0
                        nc.sync.dma_start(self8_sb[:, ga:gb, :],
                                          sel[:, sg0 + ga:sg0 + gb, :])
                        nc.sync.dma_start(g_sb[:, ga:gb, :],
                                          stream[:, sg0 + ga:sg0 + gb, :])
                else:
                    nc.sync.dma_start(g_sb[:, :sg1 - sg0, :],
                                      stream[:, sg0:sg1, :])
                    nc.sync.dma_start(self8_sb[:, :sg1 - sg0, :],
                                      sel[:, sg0:sg1, :])
                xso_sb = xpool.tile([P, TB, KC * P], dt.bfloat16, tag="xso")
                nc.sync.dma_start(xso_sb[:, :nb_t, :], xsoT[:, t0:t1, :])
                if bi == 0:
                    emit_consts()
                out_blk = opool.tile([P, TB, F_OUT], dt.bfloat16, tag="out_sb")

                for t in range(t0, t1):
                    ng = NG_t[t]
                    ngd = NGD_t[t]
                    goff = G_off[t] - sg0

                    # selection: aggT[fchunk, dst] += G_chunk^T @ selR
                    aggT_ps = psA.tile([P, F], dt.float32, tag="aggT_ps")
                    # e4m3 DoubleRow pairs (two groups per matmul)
                    for dpair in range(ngd // 2):
                        ga = goff + 2 * dpair
                        for c in range(KC):
                            nc.tensor.matmul(
                                aggT_ps[:, c * P:(c + 1) * P],
                                lhsT=g_sb[:, ga:ga + 2, c * P:(c + 1) * P]
                                    .bitcast(dt.float8e4),
                                rhs=self8_sb[:, ga:ga + 2, :]
                                    .bitcast(dt.float8e4),
                                start=(dpair == 0 and c == 0),
                                stop=(ng == ngd and dpair == ngd // 2 - 1
                                      and c == KC - 1),
                                perf_mode=DR,
                                skip_group_check=True,
                            )
                    # e3m4 singles
                    for g in range(ngd, ng):
                        for c in range(KC):
                            nc.tensor.matmul(
                                aggT_ps[:, c * P:(c + 1) * P],
                                lhsT=g_sb[:, goff + g, c * P:(c + 1) * P],
                                rhs=self8_sb[:, goff + g, :],
                                start=(ngd == 0 and g == 0 and c == 0),
                                stop=(g == ng - 1 and c == KC - 1),
                                skip_group_check=True,
                            )

                    # PSUM -> SBUF copy with the self-loop term fused in
                    aggT_sb = aggpool.tile([P, F], dt.bfloat16, tag="aggT_sb")
                    nc.vector.tensor_tensor(
                        aggT_sb[:],
                        aggT_ps[:],
                        xso_sb[:, t - t0, :],
                        mybir.AluOpType.add,
                    )

                    if prev is not None:
                        emit_tail(*prev)
                    prev = (t, aggT_sb, out_blk, t - t0,
                            (t0, t1) if t == t1 - 1 else None)

            emit_tail(*prev)

    nc.compile()
    return nc


_CACHE = {}


def _get_program(meta):
    key = (meta["N"], meta["F"], meta["F_OUT"], meta["TOT"], meta["G_TOT"],
           tuple(meta["NG_t"]), tuple(meta["NGD_t"]))
    if key not in _CACHE:
        _CACHE[key] = _build_program(meta)
    return _CACHE[key]


def kernel(x, edge_index, W, b, gamma, beta, running_mean, running_var,
           _want_results_holder=None, _run_kwargs=None):
    meta, in_maps = _prep(x, edge_index, W, b, gamma, beta,
                          running_mean, running_var)
    nc = _get_program(meta)

    from concourse.bass_utils import run_bass_kernel_spmd

    res = run_bass_kernel_spmd(nc, in_maps, core_ids=list(range(N_CORES)),
                               **(_run_kwargs or {}))
    if _want_results_holder is not None:
        _want_results_holder.append((nc, meta, in_maps, res))

    T, F_OUT = meta["T"], meta["F_OUT"]
    node_map = meta["node_map"]
    out = np.empty((meta["N"], F_OUT), dtype=np.float32)
    for k in range(N_CORES):
        tiled = np.asarray(res.results[k]["out"], dtype=np.float32)  # [128, T, F_OUT]
        rows = np.ascontiguousarray(tiled.transpose(1, 0, 2))  # [T, 128, F]
        nm = node_map[k]
        valid = nm >= 0
        out[nm[valid]] = rows[valid]
    return out
